# revision 27
# baseline (speedup 1.0000x reference)
"""Multi-head attention + residual + batchnorm on 8 trn2 NeuronCores.

Sharding: core c handles batch b = c % 4 and head-group g = c // 4
(4 heads = 512 feature dims per group). The PE clock on this part is
GPIO-throttled to ~1.95 GHz, so matmul COUNT is the currency: every
contraction >= 256 deep runs as fp8e4m3 DoubleRow (2 MACs/cell/cycle),
halving the matmul count for the QKV projections (contract D=1024), the
attention*V chain (contract S=2048) and the softmax denominator (ones
matmul). Scores contract only d_head=128 and stay bf16.

Weights and activations are pre-scaled by 32 on the host so fp8e4m3
operates in its normal range; the scale falls out via the exp argument
(1/32768) and a 32-valued ones stationary for the denominator.

  KT[u,t] = Wk_g' @ keys[b].T          (fp8 DR, psum f32 -> bf16)
  V[t,u]  = keys[b] @ Wv_g'.T          (fp8 DR, -> fp8)
  QT[u,t] = Wq_g' @ query[b].T         (fp8 DR; 2 chunks up front, the
            rest interleaved as PE filler in the attention units)
  ST[k,q] = KT_h.T-contract QT_h       (bf16)
  PT      = exp(ST/32768)              (ACT, PSUM->fp8 SBUF; the exp
            stream paces phase 2 -- one-stage software pipeline)
  OT[u,q] = sum_k V[k,u]*PT[k,q]       (fp8 DR over k-pairs)
  32r[q]  = ones32-DR-matmul over PT   (fp8 DR; replaces a DVE tree)
  o_res   = OT/(32r) + query[b].T      (f32)
  batchnorm over (b,s): local sums + 4-core AllReduce, then affine.

Units run h-outer so each head's stats AllReduce overlaps later heads'
compute; the sqrt/affine half of finalize is deferred two units so the
scalar queue never waits on a collective.
"""
import sys

sys.path.insert(0, "/opt/trn_rl_repo")

import ml_dtypes
import numpy as np

import concourse.bass as bass
import concourse.tile as tile
from concourse import bacc, mybir
from concourse.bass_utils import run_bass_kernel_spmd

F32 = mybir.dt.float32
BF16 = mybir.dt.bfloat16
FP8 = mybir.dt.float8e4
AF = mybir.ActivationFunctionType
DR = mybir.MatmulPerfMode.DoubleRow

B, S, D, H = 4, 2048, 1024, 8
DH = D // H          # 128
HG = 4               # heads per group (per core)
GF = HG * DH         # 512 features per group
EPS = 1e-5
P = 128
DT2 = 4              # d-tile PAIRS (D / 256)
TC = 4               # token chunks of 512
TCW = 512
KT_N = 16            # k tiles of 128 per sequence
KP_N = 8             # k tile pairs
NTOK = B * S
WSCALE = 32.0        # host pre-scale on Wq/Wk/Wv for fp8 range
ESCALE = 1.0 / (WSCALE * WSCALE * float(np.sqrt(np.float32(D))))


def _build():
    nc = bacc.Bacc(num_swdge_queues=4)
    xq = nc.declare_dram_parameter("xq", [TC, DT2, P, 2, TCW], FP8, isOutput=False)
    xk = nc.declare_dram_parameter("xk", [TC, DT2, P, 2, TCW], FP8, isOutput=False)
    wq = nc.declare_dram_parameter("wq", [DT2, P, 2, GF], FP8, isOutput=False)
    wk = nc.declare_dram_parameter("wk", [DT2, P, 2, GF], FP8, isOutput=False)
    wv = nc.declare_dram_parameter("wv", [DT2, P, 2, GF], FP8, isOutput=False)
    qres = nc.declare_dram_parameter("qres", [HG, TC, P, TCW], F32, isOutput=False)
    gamma = nc.declare_dram_parameter("gamma", [P, HG], F32, isOutput=False)
    beta = nc.declare_dram_parameter("beta", [P, HG], F32, isOutput=False)
    out = nc.declare_dram_parameter("out", [P, HG, S], F32, isOutput=True)

    with tile.TileContext(nc) as tc:
        with (
            tc.tile_pool(name="persist", bufs=1) as persist,
            tc.tile_pool(name="dram", bufs=1, space="DRAM") as dram,
        ):
            # ---- persistent SBUF ----
            QT = persist.tile([P, HG, S], BF16)          # (dh, h, q)
            KTb = persist.tile([P, HG, S], BF16)         # (dh, h, k)
            V = persist.tile([P, KT_N, GF], FP8)         # (t128, kt, u)
            o_res = persist.tile([P, HG, S], F32)
            gam = persist.tile([P, HG], F32)
            bet = persist.tile([P, HG], F32)
            ones_f = persist.tile([P, 2, P], F32)
            ones32 = persist.tile([P, 2, P], FP8)
            ones_b16 = persist.tile([P, P], BF16)
            eps_t = persist.tile([P, 1], F32)
            negh_t = persist.tile([P, 1], F32)
            cc_in = [dram.tile([P, 2], F32, name=f"cc_in{h}") for h in range(HG)]
            cc_out = [dram.tile([P, 2], F32, name=f"cc_out{h}") for h in range(HG)]

            nc.vector.memset(eps_t[:], float(EPS))
            nc.vector.memset(negh_t[:], -0.5)
            nc.vector.memset(ones_f[:], float(WSCALE))
            nc.vector.tensor_copy(ones32[:], ones_f[:])
            nc.vector.tensor_copy(ones_b16[:], ones_f[:, 0, :])

            with (
                tc.tile_pool(name="wpool", bufs=1) as wpool,
                tc.tile_pool(name="xkpool", bufs=16) as xkpool,
                tc.tile_pool(name="xqpool", bufs=16) as xqpool,
                tc.tile_pool(name="pt_pool", bufs=2) as pt_pool,
                tc.tile_pool(name="s1_pool", bufs=2) as s1_pool,
                tc.tile_pool(name="qr_pool", bufs=3) as qr_pool,
                tc.tile_pool(name="small", bufs=4) as small,
            ):
                wk_s = [wpool.tile([P, 2, GF], FP8, name=f"wk{d}") for d in range(DT2)]
                wv_s = [wpool.tile([P, 2, GF], FP8, name=f"wv{d}") for d in range(DT2)]
                wq_s = [wpool.tile([P, 2, GF], FP8, name=f"wq{d}") for d in range(DT2)]
                for d in range(DT2):
                    nc.scalar.dma_start(wk_s[d][:], wk[d])

                def load_x(pool, src, tci, tag, engs=(nc.sync, nc.gpsimd)):
                    tiles = []
                    for d in range(DT2):
                        t = pool.tile([P, 2, TCW], FP8, tag=tag)
                        engs[d % len(engs)].dma_start(t[:], src[tci, d])
                        tiles.append(t)
                    return tiles

                xq_chunks = {}

                # ---- phase 1: K^T and V projections ----
                with tc.tile_pool(name="ppsum", bufs=2, space="PSUM") as ppsum:
                    # prefetch every keys chunk up front: removes per-chunk
                    # arrival stalls (x stays resident, 16KB/partition)
                    xk_chunks = [load_x(xkpool, xk, t, "xk") for t in range(TC)]
                    for tci in range(TC):
                        xh = xk_chunks[tci]
                        if tci == 0:
                            # after chunk-0 x is queued: wv (first V use
                            # ~7us in) then the small gamma/beta tiles
                            for d in range(DT2):
                                nc.gpsimd.dma_start(wv_s[d][:], wv[d])
                            nc.scalar.dma_start(gam[:], gamma[:])
                            nc.scalar.dma_start(bet[:], beta[:])
                        elif tci == TC - 1:
                            # Q chunks (0,0)/(0,1) BEFORE the last KV chunk
                            # so they're off the critical path to unit 0
                            for cqi in (0, 1):
                                ps = ppsum.tile([P, TCW], F32, tag="pp")
                                for d in range(DT2):
                                    nc.tensor.matmul(
                                        ps[:],
                                        wq_s[d][:, :, bass.ts(0, DH)],
                                        xq_chunks[cqi][d][:],
                                        start=(d == 0),
                                        stop=(d == DT2 - 1),
                                        perf_mode=DR,
                                    )
                                nc.vector.tensor_copy(
                                    QT[:, 0, bass.ts(cqi, TCW)], ps[:]
                                )
                        for h in range(HG):
                            ps = ppsum.tile([P, TCW], F32, tag="pp")
                            for d in range(DT2):
                                nc.tensor.matmul(
                                    ps[:],
                                    wk_s[d][:, :, bass.ts(h, DH)],
                                    xh[d][:],
                                    start=(d == 0),
                                    stop=(d == DT2 - 1),
                                    perf_mode=DR,
                                )
                            nc.vector.tensor_copy(
                                KTb[:, h, bass.ts(tci, TCW)], ps[:]
                            )
                        for sub in range(TCW // P):
                            ps = ppsum.tile([P, GF], F32, tag="pp")
                            for d in range(DT2):
                                nc.tensor.matmul(
                                    ps[:],
                                    xh[d][:, :, bass.ts(sub, P)],
                                    wv_s[d][:],
                                    start=(d == 0),
                                    stop=(d == DT2 - 1),
                                    perf_mode=DR,
                                )
                            nc.vector.tensor_copy(
                                V[:, tci * (TCW // P) + sub, :], ps[:]
                            )
                        if tci == 1:
                            for d in range(DT2):
                                nc.scalar.dma_start(wq_s[d][:], wq[d])
                        elif tci == 2:
                            xq_chunks[0] = load_x(
                                xqpool, xq, 0, "xq", engs=(nc.scalar,)
                            )
                            xq_chunks[1] = load_x(
                                xqpool, xq, 1, "xq", engs=(nc.scalar,)
                            )
                        elif tci == 3:
                            xq_chunks[2] = load_x(
                                xqpool, xq, 2, "xq", engs=(nc.scalar,)
                            )
                            xq_chunks[3] = load_x(
                                xqpool, xq, 3, "xq", engs=(nc.scalar,)
                            )



                # ---- phase 2: attention, h-outer, 1-stage pipeline ----
                with (
                    tc.tile_pool(name="spsum", bufs=2, space="PSUM") as spsum,
                    tc.tile_pool(name="opsum", bufs=1, space="PSUM") as opsum,
                ):
                    bstats = [
                        small.tile(
                            [P, TC, nc.vector.BN_STATS_DIM],
                            F32,
                            name=f"bstat{h}",
                            tag=f"bstat{h}",
                            bufs=1,
                        )
                        for h in range(HG)
                    ]

                    def emit_scores(h, qi, chain):
                        """Score MMs + exps for unit (h, qi); `chain` is the
                        (head, chunk) Q-projection to interleave, or None."""
                        PT = pt_pool.tile([P, KT_N, TCW], FP8, tag="pt")
                        ps_q = None
                        if chain is not None:
                            ch, cqi = chain
                            ps_q = opsum.tile([P, TCW], F32, tag="q", bufs=2)
                            xtiles = xq_chunks[cqi]
                        for kp in range(KT_N // 2):
                            ps_s = spsum.tile([P, 2, TCW], F32, tag="s")
                            for j in range(2):
                                nc.tensor.matmul(
                                    ps_s[:, j, :],
                                    KTb[:, h, bass.ts(2 * kp + j, P)],
                                    QT[:, h, bass.ts(qi, TCW)],
                                    start=True,
                                    stop=True,
                                )
                            if ps_q is not None and kp % 2 == 0:
                                d = kp // 2
                                nc.tensor.matmul(
                                    ps_q[:],
                                    wq_s[d][:, :, bass.ts(ch, DH)],
                                    xtiles[d][:],
                                    start=(d == 0),
                                    stop=(d == DT2 - 1),
                                    perf_mode=DR,
                                    skip_group_check=True,
                                )
                            nc.scalar.activation(
                                out=PT[:, 2 * kp : 2 * kp + 2, :],
                                in_=ps_s[:],
                                func=AF.Exp,
                                scale=float(ESCALE),
                            )
                        qres_ch = qr_pool.tile([P, TCW], F32, tag="qres")
                        nc.sync.dma_start(qres_ch[:], qres[h, qi])
                        return (h, qi, PT, ps_q, chain, qres_ch)

                    def emit_tail(st):
                        h, qi, PT, ps_q, chain, qres_ch = st
                        # softmax denominator: gpsimd takes the big fp8 adds,
                        # DVE finishes in bf16, one bf16 ones-matmul sums
                        # partitions -- keeps 8 DR matmuls off the PE
                        s1 = s1_pool.tile([P, 8, TCW], BF16, tag="s1")
                        nc.gpsimd.tensor_add(
                            s1[:, 0:4, :], PT[:, 0:4, :], PT[:, 4:8, :]
                        )
                        nc.gpsimd.tensor_add(
                            s1[:, 4:8, :], PT[:, 8:12, :], PT[:, 12:16, :]
                        )
                        nc.vector.tensor_add(
                            s1[:, 0:4, :], s1[:, 0:4, :], s1[:, 4:8, :]
                        )
                        nc.vector.tensor_add(
                            s1[:, 0:2, :], s1[:, 0:2, :], s1[:, 2:4, :]
                        )
                        nc.vector.tensor_add(
                            s1[:, 0, :], s1[:, 0, :], s1[:, 1, :]
                        )
                        ps_o = opsum.tile([P, TCW], F32, tag="o", bufs=1)
                        for k in range(KP_N):
                            nc.tensor.matmul(
                                ps_o[:],
                                V[:, 2 * k : 2 * k + 2, bass.ts(h, DH)],
                                PT[:, 2 * k : 2 * k + 2, :],
                                start=(k == 0),
                                stop=(k == KP_N - 1),
                                perf_mode=DR,
                            )
                        ps_r = opsum.tile([P, TCW], F32, tag="r", bufs=1)
                        nc.tensor.matmul(
                            ps_r[:], ones_b16[:], s1[:, 0, :],
                            start=True, stop=True,
                        )
                        rb = small.tile([P, TCW], F32, tag="rb", bufs=2)
                        nc.vector.reciprocal_approx_fast(out=rb[:], in_=ps_r[:])
                        dst = o_res[:, h, bass.ts(qi, TCW)]
                        nc.vector.tensor_tensor(
                            dst, ps_o[:], rb[:], mybir.AluOpType.mult
                        )
                        nc.vector.tensor_add(dst, dst, qres_ch[:])
                        nc.vector.bn_stats(out=bstats[h][:, qi, :], in_=dst)
                        if ps_q is not None:
                            ch, cqi = chain
                            nc.vector.tensor_copy(
                                QT[:, ch, bass.ts(cqi, TCW)], ps_q[:]
                            )

                    def finalize_a(h):
                        """Local stats + AllReduce launch (no ACT involved)."""
                        mv = small.tile([P, 2], F32, tag="mv")
                        nc.vector.bn_aggr(out=mv[:], in_=bstats[h][:])
                        sh = small.tile([P, 2], F32, tag="sh")
                        nc.vector.tensor_scalar_mul(sh[:, 0:1], mv[:, 0:1], float(S))
                        sq = small.tile([P, 1], F32, tag="sq")
                        nc.vector.tensor_mul(sq[:], mv[:, 0:1], mv[:, 0:1])
                        nc.vector.tensor_add(sq[:], sq[:], mv[:, 1:2])
                        nc.vector.tensor_scalar_mul(sh[:, 1:2], sq[:], float(S))
                        nc.gpsimd.dma_start(cc_in[h][:], sh[:])
                        nc.gpsimd.collective_compute(
                            "AllReduce",
                            mybir.AluOpType.add,
                            ins=[cc_in[h].opt()],
                            outs=[cc_out[h].opt()],
                            replica_groups=[[0, 1, 2, 3], [4, 5, 6, 7]],
                        )
                        gstat = small.tile([P, 2], F32, tag=f"gstat{h}", bufs=1)
                        nc.gpsimd.dma_start(gstat[:], cc_out[h][:])
                        return gstat

                    def finalize_b(h, gstat, chunked):
                        """Global stats -> affine + writeout; emitted >= 2
                        units after finalize_a so the scalar-queue Sqrt
                        never waits on the collective."""
                        mean = small.tile([P, 1], F32, tag="mean")
                        var = small.tile([P, 1], F32, tag="var")
                        nc.vector.tensor_scalar_mul(
                            mean[:], gstat[:, 0:1], 1.0 / NTOK
                        )
                        nc.vector.tensor_scalar_mul(
                            var[:], gstat[:, 1:2], 1.0 / NTOK
                        )
                        msq = small.tile([P, 1], F32, tag="msq")
                        nc.vector.tensor_mul(msq[:], mean[:], mean[:])
                        nc.vector.tensor_sub(var[:], var[:], msq[:])
                        nc.vector.tensor_add(var[:], var[:], eps_t[:])
                        # rsqrt via gpsimd pow: keeps the scalar queue free
                        # of table switches away from the EXP set
                        rstd = small.tile([P, 1], F32, tag="rstd")
                        nc.gpsimd.tensor_tensor(
                            rstd[:], var[:], negh_t[:], mybir.AluOpType.pow
                        )
                        scale = small.tile([P, 1], F32, tag="scale")
                        shift = small.tile([P, 1], F32, tag="shift")
                        nc.vector.tensor_mul(scale[:], gam[:, h : h + 1], rstd[:])
                        nc.vector.tensor_mul(shift[:], mean[:], scale[:])
                        nc.vector.tensor_sub(shift[:], bet[:, h : h + 1], shift[:])
                        if not chunked:
                            nc.vector.tensor_scalar(
                                o_res[:, h, :],
                                o_res[:, h, :],
                                scale[:],
                                shift[:],
                                mybir.AluOpType.mult,
                                mybir.AluOpType.add,
                            )
                            nc.sync.dma_start(out[:, h, :], o_res[:, h, :])
                        else:
                            for c in range(TC):
                                seg = o_res[:, h, bass.ts(c, TCW)]
                                nc.vector.tensor_scalar(
                                    seg,
                                    seg,
                                    scale[:],
                                    shift[:],
                                    mybir.AluOpType.mult,
                                    mybir.AluOpType.add,
                                )
                                eng = nc.sync if c % 2 == 0 else nc.scalar
                                eng.dma_start(out[:, h, bass.ts(c, TCW)], seg)

                    units = [(h, qi) for h in range(HG) for qi in range(TC)]
                    pend = None
                    fin_due = []          # (due_idx, h, gstat)
                    for idx, (h, qi) in enumerate(units):
                        chain = units[idx + 2] if idx + 2 < len(units) else None
                        st = emit_scores(h, qi, chain)
                        if pend is not None:
                            emit_tail(pend)
                            if pend[1] == TC - 1:
                                gstat = finalize_a(pend[0])
                                fin_due.append((idx + 7, pend[0], gstat))
                        pend = st
                        while fin_due and fin_due[0][0] <= idx:
                            _, fh, fg = fin_due.pop(0)
                            finalize_b(fh, fg, chunked=False)
                    emit_tail(pend)
                    gstat = finalize_a(pend[0])
                    fin_due.append((10**9, pend[0], gstat))
                    for i, (_, fh, fg) in enumerate(fin_due):
                        finalize_b(fh, fg, chunked=(i == len(fin_due) - 1))

    nc.finalize()
    return nc


_NC = None


def _get_nc():
    global _NC
    if _NC is None:
        _NC = _build()
    return _NC


def _make_in_maps(query, keys, Wq, Wk, Wv, gamma, beta):
    query = np.asarray(query, dtype=np.float32)
    keys = np.asarray(keys, dtype=np.float32)
    Wq = np.asarray(Wq, dtype=np.float32)
    Wk = np.asarray(Wk, dtype=np.float32)
    Wv = np.asarray(Wv, dtype=np.float32)
    gamma = np.asarray(gamma, dtype=np.float32)
    beta = np.asarray(beta, dtype=np.float32)

    F8 = ml_dtypes.float8_e4m3
    in_maps = []
    for c in range(8):
        b, g = c % B, c // B
        rows = slice(GF * g, GF * (g + 1))
        qt = np.ascontiguousarray(query[b].T)              # (D, S)
        kt = np.ascontiguousarray(keys[b].T)

        def tilex(x):  # (D, S) -> (TC, DT2, P, 2, TCW): pair d-tiles
            return np.ascontiguousarray(
                x.reshape(DT2, 2, P, TC, TCW).transpose(3, 0, 2, 1, 4)
            ).astype(F8)

        def tilew(w):  # (D, GF) -> (DT2, P, 2, GF): pair d-tiles
            return np.ascontiguousarray(
                (w * WSCALE).reshape(DT2, 2, P, GF).transpose(0, 2, 1, 3)
            ).astype(F8)

        qres_f = qt[rows]                                   # (GF, S)
        qres4 = np.ascontiguousarray(
            qres_f.reshape(HG, P, TC, TCW).transpose(0, 2, 1, 3)
        )
        in_maps.append(
            {
                "xq": tilex(qt),
                "xk": tilex(kt),
                "wq": tilew(np.ascontiguousarray(Wq[rows].T)),
                "wk": tilew(np.ascontiguousarray(Wk[rows].T)),
                "wv": tilew(np.ascontiguousarray(Wv[rows].T)),
                "qres": qres4,
                "gamma": np.ascontiguousarray(
                    gamma[rows].reshape(HG, P).T
                ),
                "beta": np.ascontiguousarray(beta[rows].reshape(HG, P).T),
            }
        )
    return in_maps


def _run(in_maps, trace=False, **kw):
    nc = _get_nc()
    return run_bass_kernel_spmd(
        nc, in_maps, core_ids=list(range(8)), trace=trace, **kw
    )


def kernel(query, keys, Wq, Wk, Wv, gamma, beta):
    in_maps = _make_in_maps(query, keys, Wq, Wk, Wv, gamma, beta)
    res = _run(in_maps)
    output = np.empty((B, S, D), dtype=np.float32)
    for c in range(8):
        b, g = c % B, c // B
        oc = res.results[c]["out"]                   # (128, 4, 2048)
        block = oc.transpose(2, 1, 0).reshape(S, GF)  # (S, GF): [t, h*128+p]
        output[b, :, GF * g : GF * (g + 1)] = block
    return output


# revision 28
# speedup vs baseline: 1.4356x; 1.4356x over previous
"""Multi-head attention + residual + batchnorm on 8 trn2 NeuronCores.

Sharding: core c handles batch b = c % 4 and head-group g = c // 4
(4 heads = 512 feature dims per group). The PE clock on this part is
GPIO-throttled to ~1.95 GHz, so matmul COUNT is the currency: every
contraction >= 256 deep runs as fp8e4m3 DoubleRow (2 MACs/cell/cycle),
halving the matmul count for the QKV projections (contract D=1024), the
attention*V chain (contract S=2048) and the softmax denominator (ones
matmul). Scores contract only d_head=128 and stay bf16.

Weights and activations are pre-scaled by 32 on the host so fp8e4m3
operates in its normal range; the scale falls out via the exp argument
(1/32768) and a 32-valued ones stationary for the denominator.

  KT[u,t] = Wk_g' @ keys[b].T          (fp8 DR, psum f32 -> bf16)
  V[t,u]  = keys[b] @ Wv_g'.T          (fp8 DR, -> fp8)
  QT[u,t] = Wq_g' @ query[b].T         (fp8 DR; 2 chunks up front, the
            rest interleaved as PE filler in the attention units)
  ST[k,q] = KT_h.T-contract QT_h       (bf16)
  PT      = exp(ST/32768)              (ACT, PSUM->fp8 SBUF; the exp
            stream paces phase 2 -- one-stage software pipeline)
  OT[u,q] = sum_k V[k,u]*PT[k,q]       (fp8 DR over k-pairs)
  32r[q]  = ones32-DR-matmul over PT   (fp8 DR; replaces a DVE tree)
  o_res   = OT/(32r) + query[b].T      (f32)
  batchnorm over (b,s): local sums + 4-core AllReduce, then affine.

Units run h-outer so each head's stats AllReduce overlaps later heads'
compute; the sqrt/affine half of finalize is deferred two units so the
scalar queue never waits on a collective.
"""
import sys

sys.path.insert(0, "/opt/trn_rl_repo")

import ml_dtypes
import numpy as np

import concourse.bass as bass
import concourse.tile as tile
from concourse import bacc, mybir
from concourse.bass_utils import run_bass_kernel_spmd

F32 = mybir.dt.float32
BF16 = mybir.dt.bfloat16
FP8 = mybir.dt.float8e4
AF = mybir.ActivationFunctionType
DR = mybir.MatmulPerfMode.DoubleRow

B, S, D, H = 4, 2048, 1024, 8
DH = D // H          # 128
HG = 4               # heads per group (per core)
GF = HG * DH         # 512 features per group
EPS = 1e-5
P = 128
DT2 = 4              # d-tile PAIRS (D / 256)
TC = 4               # token chunks of 512
TCW = 512
KT_N = 16            # k tiles of 128 per sequence
KP_N = 8             # k tile pairs
NTOK = B * S
WSCALE = 32.0        # host pre-scale on Wq/Wk/Wv for fp8 range
ESCALE = 1.0 / (WSCALE * WSCALE * float(np.sqrt(np.float32(D))))


def _build():
    nc = bacc.Bacc(num_swdge_queues=4)
    xq = nc.declare_dram_parameter("xq", [TC, DT2, P, 2, TCW], FP8, isOutput=False)
    xk = nc.declare_dram_parameter("xk", [TC, DT2, P, 2, TCW], FP8, isOutput=False)
    wq = nc.declare_dram_parameter("wq", [DT2, P, 2, GF], FP8, isOutput=False)
    wk = nc.declare_dram_parameter("wk", [DT2, P, 2, GF], FP8, isOutput=False)
    wv = nc.declare_dram_parameter("wv", [DT2, P, 2, GF], FP8, isOutput=False)
    qres = nc.declare_dram_parameter("qres", [HG, TC, P, TCW], F32, isOutput=False)
    gamma = nc.declare_dram_parameter("gamma", [P, HG], F32, isOutput=False)
    beta = nc.declare_dram_parameter("beta", [P, HG], F32, isOutput=False)
    out = nc.declare_dram_parameter("out", [P, HG, S], F32, isOutput=True)

    with tile.TileContext(nc) as tc:
        with (
            tc.tile_pool(name="persist", bufs=1) as persist,
            tc.tile_pool(name="dram", bufs=1, space="DRAM") as dram,
        ):
            # ---- persistent SBUF ----
            QT = persist.tile([P, HG, S], BF16)          # (dh, h, q)
            KTb = persist.tile([P, HG, S], BF16)         # (dh, h, k)
            V = persist.tile([P, KT_N, GF], FP8)         # (t128, kt, u)
            o_res = persist.tile([P, HG, S], F32)
            gam = persist.tile([P, HG], F32)
            bet = persist.tile([P, HG], F32)
            ones_f = persist.tile([P, 2, P], F32)
            ones32 = persist.tile([P, 2, P], FP8)
            eps_t = persist.tile([P, 1], F32)
            negh_t = persist.tile([P, 1], F32)
            cc_in = [dram.tile([P, 2], F32, name=f"cc_in{h}") for h in range(HG)]
            cc_out = [dram.tile([P, 2], F32, name=f"cc_out{h}") for h in range(HG)]

            nc.vector.memset(eps_t[:], float(EPS))
            nc.vector.memset(negh_t[:], -0.5)
            nc.vector.memset(ones_f[:], float(WSCALE))
            nc.vector.tensor_copy(ones32[:], ones_f[:])

            with (
                tc.tile_pool(name="wpool", bufs=1) as wpool,
                tc.tile_pool(name="xkpool", bufs=16) as xkpool,
                tc.tile_pool(name="xqpool", bufs=16) as xqpool,
                tc.tile_pool(name="pt_pool", bufs=2) as pt_pool,
                tc.tile_pool(name="qr_pool", bufs=3) as qr_pool,
                tc.tile_pool(name="small", bufs=4) as small,
            ):
                wk_s = [wpool.tile([P, 2, GF], FP8, name=f"wk{d}") for d in range(DT2)]
                wv_s = [wpool.tile([P, 2, GF], FP8, name=f"wv{d}") for d in range(DT2)]
                wq_s = [wpool.tile([P, 2, GF], FP8, name=f"wq{d}") for d in range(DT2)]
                for d in range(DT2):
                    nc.scalar.dma_start(wk_s[d][:], wk[d])

                def load_x(pool, src, tci, tag, engs=(nc.sync, nc.gpsimd)):
                    tiles = []
                    for d in range(DT2):
                        t = pool.tile([P, 2, TCW], FP8, tag=tag)
                        engs[d % len(engs)].dma_start(t[:], src[tci, d])
                        tiles.append(t)
                    return tiles

                xq_chunks = {}

                # ---- phase 1: K^T and V projections ----
                with tc.tile_pool(name="ppsum", bufs=2, space="PSUM") as ppsum:
                    # prefetch every keys chunk up front: removes per-chunk
                    # arrival stalls (x stays resident, 16KB/partition)
                    xk_chunks = [load_x(xkpool, xk, t, "xk") for t in range(TC)]
                    for tci in range(TC):
                        xh = xk_chunks[tci]
                        if tci == 0:
                            # after chunk-0 x is queued: wv (first V use
                            # ~7us in) then the small gamma/beta tiles
                            for d in range(DT2):
                                nc.gpsimd.dma_start(wv_s[d][:], wv[d])
                            nc.scalar.dma_start(gam[:], gamma[:])
                            nc.scalar.dma_start(bet[:], beta[:])
                        elif tci == TC - 1:
                            # Q chunks (0,0)/(0,1) BEFORE the last KV chunk
                            # so they're off the critical path to unit 0
                            for cqi in (0, 1):
                                ps = ppsum.tile([P, TCW], F32, tag="pp")
                                for d in range(DT2):
                                    nc.tensor.matmul(
                                        ps[:],
                                        wq_s[d][:, :, bass.ts(0, DH)],
                                        xq_chunks[cqi][d][:],
                                        start=(d == 0),
                                        stop=(d == DT2 - 1),
                                        perf_mode=DR,
                                    )
                                nc.vector.tensor_copy(
                                    QT[:, 0, bass.ts(cqi, TCW)], ps[:]
                                )
                        for h in range(HG):
                            ps = ppsum.tile([P, TCW], F32, tag="pp")
                            for d in range(DT2):
                                nc.tensor.matmul(
                                    ps[:],
                                    wk_s[d][:, :, bass.ts(h, DH)],
                                    xh[d][:],
                                    start=(d == 0),
                                    stop=(d == DT2 - 1),
                                    perf_mode=DR,
                                )
                            nc.vector.tensor_copy(
                                KTb[:, h, bass.ts(tci, TCW)], ps[:]
                            )
                        for sub in range(TCW // P):
                            ps = ppsum.tile([P, GF], F32, tag="pp")
                            for d in range(DT2):
                                nc.tensor.matmul(
                                    ps[:],
                                    xh[d][:, :, bass.ts(sub, P)],
                                    wv_s[d][:],
                                    start=(d == 0),
                                    stop=(d == DT2 - 1),
                                    perf_mode=DR,
                                )
                            nc.vector.tensor_copy(
                                V[:, tci * (TCW // P) + sub, :], ps[:]
                            )
                        if tci == 1:
                            for d in range(DT2):
                                nc.scalar.dma_start(wq_s[d][:], wq[d])
                        elif tci == 2:
                            xq_chunks[0] = load_x(
                                xqpool, xq, 0, "xq", engs=(nc.scalar,)
                            )
                            xq_chunks[1] = load_x(
                                xqpool, xq, 1, "xq", engs=(nc.scalar,)
                            )
                        elif tci == 3:
                            xq_chunks[2] = load_x(
                                xqpool, xq, 2, "xq", engs=(nc.scalar,)
                            )
                            xq_chunks[3] = load_x(
                                xqpool, xq, 3, "xq", engs=(nc.scalar,)
                            )



                # ---- phase 2: attention, h-outer, 1-stage pipeline ----
                with (
                    tc.tile_pool(name="spsum", bufs=2, space="PSUM") as spsum,
                    tc.tile_pool(name="opsum", bufs=1, space="PSUM") as opsum,
                ):
                    bstats = [
                        small.tile(
                            [P, TC, nc.vector.BN_STATS_DIM],
                            F32,
                            name=f"bstat{h}",
                            tag=f"bstat{h}",
                            bufs=1,
                        )
                        for h in range(HG)
                    ]

                    def emit_scores(h, qi, chain):
                        """Score MMs + exps for unit (h, qi); `chain` is the
                        (head, chunk) Q-projection to interleave, or None."""
                        PT = pt_pool.tile([P, KT_N, TCW], FP8, tag="pt")
                        ps_q = None
                        if chain is not None:
                            ch, cqi = chain
                            ps_q = opsum.tile([P, TCW], F32, tag="q", bufs=2)
                            xtiles = xq_chunks[cqi]
                        for kp in range(KT_N // 2):
                            ps_s = spsum.tile([P, 2, TCW], F32, tag="s")
                            for j in range(2):
                                nc.tensor.matmul(
                                    ps_s[:, j, :],
                                    KTb[:, h, bass.ts(2 * kp + j, P)],
                                    QT[:, h, bass.ts(qi, TCW)],
                                    start=True,
                                    stop=True,
                                )
                            if ps_q is not None and kp % 2 == 0:
                                d = kp // 2
                                nc.tensor.matmul(
                                    ps_q[:],
                                    wq_s[d][:, :, bass.ts(ch, DH)],
                                    xtiles[d][:],
                                    start=(d == 0),
                                    stop=(d == DT2 - 1),
                                    perf_mode=DR,
                                    skip_group_check=True,
                                )
                            nc.scalar.activation(
                                out=PT[:, 2 * kp : 2 * kp + 2, :],
                                in_=ps_s[:],
                                func=AF.Exp,
                                scale=float(ESCALE),
                            )
                        qres_ch = qr_pool.tile([P, TCW], F32, tag="qres")
                        nc.sync.dma_start(qres_ch[:], qres[h, qi])
                        return (h, qi, PT, ps_q, chain, qres_ch)

                    def emit_tail(st):
                        h, qi, PT, ps_q, chain, qres_ch = st
                        ps_o = opsum.tile([P, TCW], F32, tag="o", bufs=1)
                        for k in range(KP_N):
                            nc.tensor.matmul(
                                ps_o[:],
                                V[:, 2 * k : 2 * k + 2, bass.ts(h, DH)],
                                PT[:, 2 * k : 2 * k + 2, :],
                                start=(k == 0),
                                stop=(k == KP_N - 1),
                                perf_mode=DR,
                            )
                        ps_r = opsum.tile([P, TCW], F32, tag="r", bufs=1)
                        for k in range(KP_N):
                            m = nc.tensor.matmul(
                                ps_r[:],
                                ones32[:],
                                PT[:, 2 * k : 2 * k + 2, :],
                                start=(k == 0),
                                stop=(k == KP_N - 1),
                                perf_mode=DR,
                            )
                            if k > 0:
                                # same stationary as k==0: reuse the loaded
                                # weights instead of re-issuing LDWEIGHTS
                                m.ins.ldweights = False
                        rb = small.tile([P, TCW], F32, tag="rb", bufs=2)
                        nc.vector.reciprocal_approx_fast(out=rb[:], in_=ps_r[:])
                        dst = o_res[:, h, bass.ts(qi, TCW)]
                        nc.vector.tensor_tensor(
                            dst, ps_o[:], rb[:], mybir.AluOpType.mult
                        )
                        nc.vector.tensor_add(dst, dst, qres_ch[:])
                        nc.vector.bn_stats(out=bstats[h][:, qi, :], in_=dst)
                        if ps_q is not None:
                            ch, cqi = chain
                            nc.vector.tensor_copy(
                                QT[:, ch, bass.ts(cqi, TCW)], ps_q[:]
                            )

                    def finalize_a(h):
                        """Local stats + AllReduce launch (no ACT involved)."""
                        mv = small.tile([P, 2], F32, tag="mv")
                        nc.vector.bn_aggr(out=mv[:], in_=bstats[h][:])
                        sh = small.tile([P, 2], F32, tag="sh")
                        nc.vector.tensor_scalar_mul(sh[:, 0:1], mv[:, 0:1], float(S))
                        sq = small.tile([P, 1], F32, tag="sq")
                        nc.vector.tensor_mul(sq[:], mv[:, 0:1], mv[:, 0:1])
                        nc.vector.tensor_add(sq[:], sq[:], mv[:, 1:2])
                        nc.vector.tensor_scalar_mul(sh[:, 1:2], sq[:], float(S))
                        nc.gpsimd.dma_start(cc_in[h][:], sh[:])
                        nc.gpsimd.collective_compute(
                            "AllReduce",
                            mybir.AluOpType.add,
                            ins=[cc_in[h].opt()],
                            outs=[cc_out[h].opt()],
                            replica_groups=[[0, 1, 2, 3], [4, 5, 6, 7]],
                        )
                        gstat = small.tile([P, 2], F32, tag=f"gstat{h}", bufs=1)
                        nc.gpsimd.dma_start(gstat[:], cc_out[h][:])
                        return gstat

                    def finalize_b(h, gstat, chunked):
                        """Global stats -> affine + writeout; emitted >= 2
                        units after finalize_a so the scalar-queue Sqrt
                        never waits on the collective."""
                        mean = small.tile([P, 1], F32, tag="mean")
                        var = small.tile([P, 1], F32, tag="var")
                        nc.vector.tensor_scalar_mul(
                            mean[:], gstat[:, 0:1], 1.0 / NTOK
                        )
                        nc.vector.tensor_scalar_mul(
                            var[:], gstat[:, 1:2], 1.0 / NTOK
                        )
                        msq = small.tile([P, 1], F32, tag="msq")
                        nc.vector.tensor_mul(msq[:], mean[:], mean[:])
                        nc.vector.tensor_sub(var[:], var[:], msq[:])
                        nc.vector.tensor_add(var[:], var[:], eps_t[:])
                        # rsqrt via gpsimd pow: keeps the scalar queue free
                        # of table switches away from the EXP set
                        rstd = small.tile([P, 1], F32, tag="rstd")
                        nc.gpsimd.tensor_tensor(
                            rstd[:], var[:], negh_t[:], mybir.AluOpType.pow
                        )
                        scale = small.tile([P, 1], F32, tag="scale")
                        shift = small.tile([P, 1], F32, tag="shift")
                        nc.vector.tensor_mul(scale[:], gam[:, h : h + 1], rstd[:])
                        nc.vector.tensor_mul(shift[:], mean[:], scale[:])
                        nc.vector.tensor_sub(shift[:], bet[:, h : h + 1], shift[:])
                        if not chunked:
                            nc.vector.tensor_scalar(
                                o_res[:, h, :],
                                o_res[:, h, :],
                                scale[:],
                                shift[:],
                                mybir.AluOpType.mult,
                                mybir.AluOpType.add,
                            )
                            nc.sync.dma_start(out[:, h, :], o_res[:, h, :])
                        else:
                            for c in range(TC):
                                seg = o_res[:, h, bass.ts(c, TCW)]
                                nc.vector.tensor_scalar(
                                    seg,
                                    seg,
                                    scale[:],
                                    shift[:],
                                    mybir.AluOpType.mult,
                                    mybir.AluOpType.add,
                                )
                                eng = nc.sync if c % 2 == 0 else nc.scalar
                                eng.dma_start(out[:, h, bass.ts(c, TCW)], seg)

                    units = [(h, qi) for h in range(HG) for qi in range(TC)]
                    pend = None
                    fin_due = []          # (due_idx, h, gstat)
                    for idx, (h, qi) in enumerate(units):
                        chain = units[idx + 2] if idx + 2 < len(units) else None
                        st = emit_scores(h, qi, chain)
                        if pend is not None:
                            emit_tail(pend)
                            if pend[1] == TC - 1:
                                gstat = finalize_a(pend[0])
                                fin_due.append((idx + 7, pend[0], gstat))
                        pend = st
                        while fin_due and fin_due[0][0] <= idx:
                            _, fh, fg = fin_due.pop(0)
                            finalize_b(fh, fg, chunked=False)
                    emit_tail(pend)
                    gstat = finalize_a(pend[0])
                    fin_due.append((10**9, pend[0], gstat))
                    for i, (_, fh, fg) in enumerate(fin_due):
                        finalize_b(fh, fg, chunked=(i == len(fin_due) - 1))

    nc.finalize()
    return nc


_NC = None


def _get_nc():
    global _NC
    if _NC is None:
        _NC = _build()
    return _NC


def _make_in_maps(query, keys, Wq, Wk, Wv, gamma, beta):
    query = np.asarray(query, dtype=np.float32)
    keys = np.asarray(keys, dtype=np.float32)
    Wq = np.asarray(Wq, dtype=np.float32)
    Wk = np.asarray(Wk, dtype=np.float32)
    Wv = np.asarray(Wv, dtype=np.float32)
    gamma = np.asarray(gamma, dtype=np.float32)
    beta = np.asarray(beta, dtype=np.float32)

    F8 = ml_dtypes.float8_e4m3
    in_maps = []
    for c in range(8):
        b, g = c % B, c // B
        rows = slice(GF * g, GF * (g + 1))
        qt = np.ascontiguousarray(query[b].T)              # (D, S)
        kt = np.ascontiguousarray(keys[b].T)

        def tilex(x):  # (D, S) -> (TC, DT2, P, 2, TCW): pair d-tiles
            return np.ascontiguousarray(
                x.reshape(DT2, 2, P, TC, TCW).transpose(3, 0, 2, 1, 4)
            ).astype(F8)

        def tilew(w):  # (D, GF) -> (DT2, P, 2, GF): pair d-tiles
            return np.ascontiguousarray(
                (w * WSCALE).reshape(DT2, 2, P, GF).transpose(0, 2, 1, 3)
            ).astype(F8)

        qres_f = qt[rows]                                   # (GF, S)
        qres4 = np.ascontiguousarray(
            qres_f.reshape(HG, P, TC, TCW).transpose(0, 2, 1, 3)
        )
        in_maps.append(
            {
                "xq": tilex(qt),
                "xk": tilex(kt),
                "wq": tilew(np.ascontiguousarray(Wq[rows].T)),
                "wk": tilew(np.ascontiguousarray(Wk[rows].T)),
                "wv": tilew(np.ascontiguousarray(Wv[rows].T)),
                "qres": qres4,
                "gamma": np.ascontiguousarray(
                    gamma[rows].reshape(HG, P).T
                ),
                "beta": np.ascontiguousarray(beta[rows].reshape(HG, P).T),
            }
        )
    return in_maps


def _run(in_maps, trace=False, **kw):
    nc = _get_nc()
    return run_bass_kernel_spmd(
        nc, in_maps, core_ids=list(range(8)), trace=trace, **kw
    )


def kernel(query, keys, Wq, Wk, Wv, gamma, beta):
    in_maps = _make_in_maps(query, keys, Wq, Wk, Wv, gamma, beta)
    res = _run(in_maps)
    output = np.empty((B, S, D), dtype=np.float32)
    for c in range(8):
        b, g = c % B, c // B
        oc = res.results[c]["out"]                   # (128, 4, 2048)
        block = oc.transpose(2, 1, 0).reshape(S, GF)  # (S, GF): [t, h*128+p]
        output[b, :, GF * g : GF * (g + 1)] = block
    return output
